# revision 14
# baseline (speedup 1.0000x reference)
"""Trainium2 Bass kernel for nn_CondRnnSampler.

Computes, for each batch row b:
    out[b] = sum_i log_softmax(MLP(h_i))[s_i]  over a 64-step LSTM scan,
with the LSTM consuming x_i = token_embed[s_i] + pos_enc(p_i).

Strategy: pure data parallel over 8 NeuronCores (512 batch rows each).
All activations are feature-major ([features-on-partitions, batch-on-free]) so
every layer is a stationary-weight matmul with N=512 moving columns. All large
matmuls run in fp8e4 with MatmulPerfMode.DoubleRow: both 128-row k-tiles of the
contraction are packed into one instruction ([K=128, 2, M] stationary against
[K=128, 2, N] moving), halving the PE instruction count vs bf16. Accumulation
stays fp32 in PSUM; the LSTM cell state c is bf16.

Per step one merged DMA delivers the host-built index encodings: the sample
one-hot (fp8, DoubleRow-packed) and the positional-encoding rows pe_table[p]
(fp8, same packing) in a single [128, 4, 512] tile. token_embed[s] is a
one-hot DoubleRow matmul; x = (te gather PSUM) + pex in one fused DVE add.
The per-step picked-logit and softmax-denominator reductions are single
DoubleRow matmuls against a sliding selector so step j lands on PSUM partition
j; exp is deferred into SUB-step blocks (one big ACTIVATE per k-tile, output
fp8e5 for range) so the ACT table set switches only twice per SUB steps.

Step order follows the proven baseline: all consumers of h_i (x-gather for
i+1, previous pick, esum drain, hid, gates, logits) are emitted before the
cell update writes h_{i+1}, keeping the recurrence chain short and the
h-independent work in front of it.
"""

import sys

sys.path.insert(0, "/opt/trn_rl_repo")

from contextlib import ExitStack

import ml_dtypes
import numpy as np

import concourse.bacc as bacc
import concourse.tile as tile
from concourse import bass_utils, mybir
from concourse.bass import ts

B, D, E, NCL = 4096, 64, 256, 256  # batch, steps, embed, n_choices
NCORES = 8
BS = B // NCORES  # 512 rows per core
P = 128
SUB = 8  # deferred-softmax block (steps)

AF = mybir.ActivationFunctionType
OP = mybir.AluOpType
F32 = mybir.dt.float32
BF16 = mybir.dt.bfloat16
FP8 = mybir.dt.float8e4
FP8E5 = mybir.dt.float8e5
DR = mybir.MatmulPerfMode.DoubleRow

NPBF = ml_dtypes.bfloat16
NPE4 = ml_dtypes.float8_e4m3
NPE5 = ml_dtypes.float8_e5m2

SIG = AF.Sigmoid
TANH = AF.Tanh
# gate-dim blocks j of 128 over 4E=1024: (i0,i1,f0,f1,g0,g1,o0,o1)
GATE_FUNCS = [SIG, SIG, SIG, SIG, TANH, TANH, SIG, SIG]


def _pe_table() -> np.ndarray:
    half = np.float32(E // 2)
    inv = (
        np.float32(1.0)
        / (np.float32(10000.0) ** (np.arange(E // 2, dtype=np.float32) / half))
    ).astype(np.float32)
    pos = np.arange(D, dtype=np.float32)[:, None]
    ang = pos * inv[None, :]
    return np.concatenate([np.sin(ang), np.cos(ang)], axis=1).astype(np.float32)


def build_bass(n_steps: int = D):
    """Build the per-core Bass program (identical on all 8 cores)."""
    nc = bacc.Bacc("TRN2", debug=False, target_bir_lowering=False, num_devices=NCORES)

    def din(name, shape, dt=FP8):
        return nc.dram_tensor(name, list(shape), dt, kind="ExternalInput").ap()

    wiht_d = din("wiht", (E, 4 * E))  # W_ih.T
    whht_d = din("whht", (E, 4 * E))  # W_hh.T
    w1t_d = din("w1t", (E, 2 * E))  # W1.T
    w2t_d = din("w2t", (2 * E, NCL))  # W2.T
    te_d = din("te", (NCL, E))  # token_embed (lhsT for the gather)
    bgc_d = din("bgc", (P, 8), F32)
    b1c_d = din("b1c", (P, 4), F32)
    b2c_d = din("b2c", (P, 2), F32)
    # per-step stream: blocks (oh_k0, oh_k1, pex_k0, pex_k1)
    strm_d = din("strm", (D, P, 4, BS))
    slide4_d = din("slide4", (P, 2, 2 * D))  # pick selector (fp8e4)
    slide5_d = din("slide5", (P, 2, 2 * D), FP8E5)  # esum selector
    ones64_d = din("ones64", (D, 1), F32)
    out_d = nc.dram_tensor("out", [1, BS], F32, kind="ExternalOutput").ap()

    with tile.TileContext(nc) as tc:
        with ExitStack() as ctx:
            sing = ctx.enter_context(tc.tile_pool(name="sing", bufs=1))
            strmq = ctx.enter_context(tc.tile_pool(name="strmq", bufs=6))
            xpool = ctx.enter_context(tc.tile_pool(name="xpool", bufs=3))
            gpool = ctx.enter_context(tc.tile_pool(name="gpool", bufs=10))
            tpool = ctx.enter_context(tc.tile_pool(name="tpool", bufs=4))
            hidp = ctx.enter_context(tc.tile_pool(name="hidp", bufs=2))
            prodp = ctx.enter_context(tc.tile_pool(name="prodp", bufs=3))
            epool = ctx.enter_context(tc.tile_pool(name="epool", bufs=2))
            pp = ctx.enter_context(tc.tile_pool(name="pp", bufs=4, space="PSUM"))
            xpp = ctx.enter_context(tc.tile_pool(name="xpp", bufs=1, space="PSUM"))
            psing = ctx.enter_context(
                tc.tile_pool(name="psing", bufs=1, space="PSUM")
            )

            # ---- streaming index-derived inputs -----------------------------
            strm_t = {}

            def fetch_stream(i):
                s = strmq.tile([P, 4, BS], FP8, tag="strm")
                nc.sync.dma_start(s[:], strm_d[i])
                strm_t[i] = s

            for _i in range(4):
                fetch_stream(_i)

            # ---- resident SBUF tensors -------------------------------------
            te2 = sing.tile([P, 2, E], FP8, tag="te")
            nc.sync.dma_start(te2[:], te_d.rearrange("(ko p) m -> p ko m", p=P))
            wiht = sing.tile([P, 2, 4 * E], FP8, tag="wiht")
            nc.sync.dma_start(wiht[:], wiht_d.rearrange("(ko p) m -> p ko m", p=P))
            bgc = sing.tile([P, 8], F32, tag="bgc")
            nc.sync.dma_start(bgc[:], bgc_d)
            whht = sing.tile([P, 2, 4 * E], FP8, tag="whht")
            nc.sync.dma_start(whht[:], whht_d.rearrange("(ko p) m -> p ko m", p=P))
            w1t = sing.tile([P, 2, 2 * E], FP8, tag="w1t")
            nc.sync.dma_start(w1t[:], w1t_d.rearrange("(ko p) m -> p ko m", p=P))
            w2t = sing.tile([P, 4, NCL], FP8, tag="w2t")
            nc.sync.dma_start(w2t[:], w2t_d.rearrange("(ko p) m -> p ko m", p=P))
            b1c = sing.tile([P, 4], F32, tag="b1c")
            nc.sync.dma_start(b1c[:], b1c_d)
            b2c = sing.tile([P, 2], F32, tag="b2c")
            nc.sync.dma_start(b2c[:], b2c_d)
            slide4 = sing.tile([P, 2, 2 * D], FP8, tag="slide4")
            nc.sync.dma_start(slide4[:], slide4_d)
            slide5 = sing.tile([P, 2, 2 * D], FP8E5, tag="slide5")
            nc.sync.dma_start(slide5[:], slide5_d)
            ones64 = sing.tile([D, 1], F32, tag="ones64")
            nc.sync.dma_start(ones64[:], ones64_d)

            h_sb = sing.tile([P, 2, BS], FP8, tag="h")
            c_sb = sing.tile([P, 2, BS], BF16, tag="c")
            lbuf = sing.tile([P, 2, 2 * SUB, BS], BF16, tag="lbuf")
            esum_ps = psing.tile([D, BS], F32, tag="esum")
            pick_ps = psing.tile([D, BS], F32, tag="pick")

            def gate(j, x_sb, with_h):
                """Pre-act matmuls + activation for gate-dim block j (of 8)."""
                g_ps = pp.tile([P, BS], F32, tag="ps")
                nc.tensor.matmul(
                    g_ps, wiht[:, :, ts(j, P)], x_sb, start=True, stop=not with_h,
                    perf_mode=DR,
                )
                if with_h:
                    nc.tensor.matmul(
                        g_ps, whht[:, :, ts(j, P)], h_sb, start=False, stop=True,
                        perf_mode=DR,
                    )
                g_sb = gpool.tile([P, BS], BF16, tag="gate")
                nc.scalar.activation(g_sb, g_ps, GATE_FUNCS[j], bias=bgc[:, j : j + 1])
                return g_sb

            def cell_k(gt, k):
                """c[:,k] = f*c + i*g; h[:,k] = o*tanh(c). gt indexed by block j.

                i*g runs on GPSIMD in parallel with f*c on DVE; the rest of the
                recurrence-critical chain (add, tanh, h mul) stays on DVE/ACT.
                """
                ig = tpool.tile([P, BS], BF16, tag="tmp")
                nc.gpsimd.tensor_mul(ig, gt[0 + k], gt[4 + k])
                fc = tpool.tile([P, BS], BF16, tag="tmp")
                nc.vector.tensor_mul(fc, gt[2 + k], c_sb[:, k, :])
                nc.vector.tensor_add(c_sb[:, k, :], ig, fc)
                tcl = tpool.tile([P, BS], BF16, tag="tc")
                nc.scalar.activation(tcl, c_sb[:, k, :], TANH)
                nc.gpsimd.tensor_mul(h_sb[:, k, :], gt[6 + k], tcl)

            pending_esum = []  # FIFO of deferred esum matmuls

            def defer_block(j0):
                """exp for steps j0..j0+SUB-1: one big ACTIVATE per k-tile."""
                blk = (j0 // SUB) % 2
                e = epool.tile([P, SUB, 2, BS], FP8E5, tag="e")
                for k in range(2):
                    nc.scalar.activation(
                        e[:, :, k, :],
                        lbuf[:, k, blk * SUB : (blk + 1) * SUB, :],
                        AF.Exp,
                    )
                for j in range(j0, j0 + SUB):
                    pending_esum.append((j, e))

            def pop_esum(n):
                for _ in range(min(n, len(pending_esum))):
                    j, e = pending_esum.pop(0)
                    nc.tensor.matmul(
                        esum_ps,
                        slide5[:, :, D - 1 - j : 2 * D - 1 - j],
                        e[:, j % SUB, :, :],
                        start=(j == 0),
                        stop=(j == n_steps - 1),
                        perf_mode=DR,
                        skip_group_check=True,
                    )

            def gather_x(i):
                """x_i = token_embed[s_i] + pe_rows, fp8 [P, 2, BS] in SBUF."""
                strm = strm_t[i]
                x_ps = xpp.tile([P, 2, BS], F32, tag="xps")
                for t in range(2):
                    nc.tensor.matmul(
                        x_ps[:, t, :], te2[:, :, ts(t, P)], strm[:, 0:2, :],
                        start=True, stop=True, perf_mode=DR,
                    )
                x_sb = xpool.tile([P, 2, BS], FP8, tag="x")
                nc.vector.tensor_add(x_sb, x_ps, strm[:, 2:4, :])
                return x_sb

            # ---- init: h,c from lstm(pe[:,0]) with zero state ---------------
            # x0 is exactly the pex half of stream 0 (fp8, DoubleRow-packed).
            gt0 = [None] * 8
            for j in (0, 4, 2, 6, 1, 5, 3, 7):
                gt0[j] = gate(j, strm_t[0][:, 2:4, :], with_h=False)
            for k in range(2):
                nc.vector.tensor_mul(c_sb[:, k, :], gt0[0 + k], gt0[4 + k])
                tcl = tpool.tile([P, BS], BF16, tag="tc")
                nc.scalar.activation(tcl, c_sb[:, k, :], TANH)
                nc.vector.tensor_mul(h_sb[:, k, :], gt0[6 + k], tcl)

            x_t = {0: gather_x(0)}
            prev_prod = None  # (step, prod pair tile) awaiting the pick matmul

            # ---- scan ------------------------------------------------------
            # Per-iteration order: h-independent PE work first (x gather for
            # i+1, previous pick, one esum drain), then hid/gates/logits (all
            # reading h_i), then the cell update writes h_{i+1} last.
            for i in range(n_steps):
                if i + 1 < n_steps:
                    x_t[i + 1] = gather_x(i + 1)
                if prev_prod is not None:
                    pj, pprod = prev_prod
                    nc.tensor.matmul(
                        pick_ps,
                        slide4[:, :, D - 1 - pj : 2 * D - 1 - pj],
                        pprod,
                        start=(pj == 0),
                        stop=False,
                        perf_mode=DR,
                        skip_group_check=True,
                    )
                    prev_prod = None
                pop_esum(1)
                if i + 4 < n_steps:
                    fetch_stream(i + 4)

                # hid = relu(W1 @ h + b1): 4 blocks (relu on DVE; PSUM source)
                hid_sb = hidp.tile([P, 4, BS], FP8, tag="hid")
                for t in range(4):
                    hp = pp.tile([P, BS], F32, tag="ps")
                    nc.tensor.matmul(
                        hp, w1t[:, :, ts(t, P)], h_sb, start=True, stop=True,
                        perf_mode=DR,
                    )
                    nc.vector.tensor_scalar(
                        hid_sb[:, t, :], hp, b1c[:, t : t + 1], 0.0, OP.add, OP.max
                    )

                # gates k0 (i,g,f,o), then the k0 cell chain while the k1
                # gate matmuls/activations run, then k1 and its cell chain.
                # hid/gates read h_i; the cell writes h_{i+1} per k-half after
                # every reader of that half has been emitted.
                gt = [None] * 8
                xi = x_t.pop(i)
                for j in (0, 4, 2, 6):
                    gt[j] = gate(j, xi, with_h=True)
                cell_k(gt, 0)
                for j in (1, 5, 3, 7):
                    gt[j] = gate(j, xi, with_h=True)
                cell_k(gt, 1)

                # logits = W2 @ hid: 2 out blocks x 2 DoubleRow each
                l_ps = []
                for t in range(2):
                    lp = pp.tile([P, BS], F32, tag="ps")
                    nc.tensor.matmul(
                        lp, w2t[:, 0:2, ts(t, P)], hid_sb[:, 0:2, :], start=True,
                        stop=False, perf_mode=DR,
                    )
                    nc.tensor.matmul(
                        lp, w2t[:, 2:4, ts(t, P)], hid_sb[:, 2:4, :], start=False,
                        stop=True, perf_mode=DR,
                    )
                    l_ps.append(lp)

                # stash l+b2 (frees the PSUM bank, pre-biased for exp and
                # pick); picked-logit product on GPSIMD, off the h path
                slot = i % (2 * SUB)
                strm_i = strm_t.pop(i)
                prods = prodp.tile([P, 2, BS], FP8, tag="prod")
                for k in range(2):
                    nc.vector.tensor_scalar(
                        lbuf[:, k, slot, :], l_ps[k], b2c[:, k : k + 1], None,
                        OP.add,
                    )
                    nc.gpsimd.tensor_mul(
                        prods[:, k, :], lbuf[:, k, slot, :], strm_i[:, k, :]
                    )
                prev_prod = (i, prods)

                if (i + 1) % SUB == 0 and i + 1 >= SUB and i + 1 < n_steps:
                    defer_block(i + 1 - SUB)

            # final pick + last deferred block + drain
            pj, pprod = prev_prod
            nc.tensor.matmul(
                pick_ps,
                slide4[:, :, D - 1 - pj : 2 * D - 1 - pj],
                pprod,
                start=False,
                stop=True,
                perf_mode=DR,
                skip_group_check=True,
            )
            defer_block(n_steps - SUB)
            pop_esum(len(pending_esum))

            # ---- epilogue: out = sum_j (pick_j - ln(esum_j)) ----------------
            ln_e = sing.tile([D, BS], F32, tag="lne")
            nc.scalar.activation(ln_e, esum_ps, AF.Ln)
            diff = sing.tile([D, BS], F32, tag="diff")
            nc.vector.tensor_sub(diff, pick_ps, ln_e)
            fin_ps = pp.tile([1, BS], F32, tag="ps")
            nc.tensor.matmul(fin_ps, ones64[:, 0:1], diff, start=True, stop=True)
            out_sb = sing.tile([1, BS], F32, tag="outsb")
            nc.scalar.activation(out_sb, fin_ps, AF.Copy)
            nc.sync.dma_start(out_d, out_sb)

    nc.compile()
    return nc


def prep_inputs(token_embed, W_ih, b_ih, W_hh, b_hh, W1, b1, W2, b2, pos_list,
                input_samples):
    """Host-side layout prep -> per-core in_maps for run_bass_kernel_spmd."""
    f = np.float32
    slide = np.zeros((P, 2, 2 * D), f)
    slide[:, :, D - 1] = 1.0
    shared = {
        "wiht": np.ascontiguousarray(np.asarray(W_ih, f).T).astype(NPE4),
        "whht": np.ascontiguousarray(np.asarray(W_hh, f).T).astype(NPE4),
        "w1t": np.ascontiguousarray(np.asarray(W1, f).T).astype(NPE4),
        "w2t": np.ascontiguousarray(np.asarray(W2, f).T).astype(NPE4),
        "te": np.asarray(token_embed, f).astype(NPE4),
        "bgc": np.ascontiguousarray(
            (np.asarray(b_ih, f) + np.asarray(b_hh, f)).reshape(8, P).T
        ),
        "b1c": np.ascontiguousarray(np.asarray(b1, f).reshape(4, P).T),
        "b2c": np.ascontiguousarray(np.asarray(b2, f).reshape(2, P).T),
        "slide4": slide.astype(NPE4),
        "slide5": slide.astype(NPE5),
        "ones64": np.ones((D, 1), f),
    }
    pe_e4 = _pe_table().astype(NPE4)  # quantize the table once, then gather
    samples = np.asarray(input_samples)
    poss = np.asarray(pos_list)
    didx = np.arange(D)[:, None]
    bidx = np.arange(BS)[None, :]
    in_maps = []
    for c in range(NCORES):
        lo, hi = c * BS, (c + 1) * BS
        sT = np.ascontiguousarray(samples[lo:hi].T)  # [D, BS]
        pT = np.ascontiguousarray(poss[lo:hi].T)
        strm = np.zeros((D, P, 4, BS), NPE4)
        strm[didx, sT % P, sT // P, bidx] = 1.0  # one-hot blocks 0:2
        # pex blocks 2:4: strm[i, p, 2+k, b] = pe_table[pos[b, i], 128k + p]
        strm[:, :, 2:4, :] = pe_e4[pT].reshape(D, BS, 2, P).transpose(0, 3, 2, 1)
        m = dict(shared)
        m["strm"] = strm
        in_maps.append(m)
    return in_maps


_CACHE = {}


def kernel(**inputs) -> np.ndarray:
    if "nc" not in _CACHE:
        _CACHE["nc"] = build_bass()
    nc = _CACHE["nc"]
    in_maps = prep_inputs(**inputs)
    res = bass_utils.run_bass_kernel_spmd(nc, in_maps, core_ids=list(range(NCORES)))
    _CACHE["last_results"] = res
    out = np.empty((B, 1), np.float32)
    for c in range(NCORES):
        out[c * BS : (c + 1) * BS, 0] = np.asarray(
            res.results[c]["out"], np.float32
        ).reshape(BS)
    return out


# revision 15
# speedup vs baseline: 1.1608x; 1.1608x over previous
"""Trainium2 Bass kernel for nn_CondRnnSampler.

Computes, for each batch row b:
    out[b] = sum_i log_softmax(MLP(h_i))[s_i]  over a 64-step LSTM scan,
with the LSTM consuming x_i = token_embed[s_i] + pos_enc(p_i).

Strategy: pure data parallel over 8 NeuronCores (512 batch rows each).
All activations are feature-major ([features-on-partitions, batch-on-free]) so
every layer is a stationary-weight matmul with N=512 moving columns. All large
matmuls run in fp8e4 with MatmulPerfMode.DoubleRow: both 128-row k-tiles of the
contraction are packed into one instruction ([K=128, 2, M] stationary against
[K=128, 2, N] moving), halving the PE instruction count vs bf16. Accumulation
stays fp32 in PSUM; the LSTM cell state c is bf16.

Per step one merged DMA delivers the host-built index encodings: the sample
one-hot (fp8, DoubleRow-packed) and the positional-encoding rows pe_table[p]
(fp8, same packing) in a single [128, 4, 512] tile. token_embed[s] is a
one-hot DoubleRow matmul; x = (te gather PSUM) + pex in one fused DVE add.
The per-step picked-logit and softmax-denominator reductions are single
DoubleRow matmuls against a sliding selector so step j lands on PSUM partition
j; exp is deferred into SUB-step blocks (one big ACTIVATE per k-tile, output
fp8e5 for range) so the ACT table set switches only twice per SUB steps.

Step order follows the proven baseline: all consumers of h_i (x-gather for
i+1, previous pick, esum drain, hid, gates, logits) are emitted before the
cell update writes h_{i+1}, keeping the recurrence chain short and the
h-independent work in front of it.
"""

import sys

sys.path.insert(0, "/opt/trn_rl_repo")

from contextlib import ExitStack

import ml_dtypes
import numpy as np

import concourse.bacc as bacc
import concourse.tile as tile
from concourse import bass_utils, mybir
from concourse.bass import ts

B, D, E, NCL = 4096, 64, 256, 256  # batch, steps, embed, n_choices
NCORES = 8
BS = B // NCORES  # 512 rows per core
P = 128
SUB = 16  # deferred-softmax block (steps)

AF = mybir.ActivationFunctionType
OP = mybir.AluOpType
F32 = mybir.dt.float32
BF16 = mybir.dt.bfloat16
FP8 = mybir.dt.float8e4
FP8E5 = mybir.dt.float8e5
DR = mybir.MatmulPerfMode.DoubleRow

NPBF = ml_dtypes.bfloat16
NPE4 = ml_dtypes.float8_e4m3
NPE5 = ml_dtypes.float8_e5m2

SIG = AF.Sigmoid
TANH = AF.Tanh
# gate-dim blocks j of 128 over 4E=1024: (i0,i1,f0,f1,g0,g1,o0,o1)
GATE_FUNCS = [SIG, SIG, SIG, SIG, TANH, TANH, SIG, SIG]


def _pe_table() -> np.ndarray:
    half = np.float32(E // 2)
    inv = (
        np.float32(1.0)
        / (np.float32(10000.0) ** (np.arange(E // 2, dtype=np.float32) / half))
    ).astype(np.float32)
    pos = np.arange(D, dtype=np.float32)[:, None]
    ang = pos * inv[None, :]
    return np.concatenate([np.sin(ang), np.cos(ang)], axis=1).astype(np.float32)


def build_bass(n_steps: int = D):
    """Build the per-core Bass program (identical on all 8 cores)."""
    nc = bacc.Bacc("TRN2", debug=False, target_bir_lowering=False, num_devices=NCORES)

    def din(name, shape, dt=FP8):
        return nc.dram_tensor(name, list(shape), dt, kind="ExternalInput").ap()

    wiht_d = din("wiht", (E, 4 * E))  # W_ih.T
    whht_d = din("whht", (E, 4 * E))  # W_hh.T
    w1t_d = din("w1t", (E, 2 * E))  # W1.T
    w2t_d = din("w2t", (2 * E, NCL))  # W2.T
    te_d = din("te", (NCL, E))  # token_embed (lhsT for the gather)
    bgc_d = din("bgc", (P, 8), F32)
    b1c_d = din("b1c", (P, 4), F32)
    b2c_d = din("b2c", (P, 2), F32)
    # per-step stream: blocks (oh_k0, oh_k1, pex_k0, pex_k1)
    strm_d = din("strm", (D, P, 4, BS))
    slide4_d = din("slide4", (P, 2, 2 * D))  # pick selector (fp8e4)
    slide5_d = din("slide5", (P, 2, 2 * D), FP8E5)  # esum selector
    ones64_d = din("ones64", (D, 1), F32)
    out_d = nc.dram_tensor("out", [1, BS], F32, kind="ExternalOutput").ap()

    with tile.TileContext(nc) as tc:
        with ExitStack() as ctx:
            sing = ctx.enter_context(tc.tile_pool(name="sing", bufs=1))
            strmq = ctx.enter_context(tc.tile_pool(name="strmq", bufs=6))
            xpool = ctx.enter_context(tc.tile_pool(name="xpool", bufs=3))
            gpool = ctx.enter_context(tc.tile_pool(name="gpool", bufs=10))
            tpool = ctx.enter_context(tc.tile_pool(name="tpool", bufs=4))
            hidp = ctx.enter_context(tc.tile_pool(name="hidp", bufs=2))
            prodp = ctx.enter_context(tc.tile_pool(name="prodp", bufs=3))
            epool = ctx.enter_context(tc.tile_pool(name="epool", bufs=2))
            pp = ctx.enter_context(tc.tile_pool(name="pp", bufs=4, space="PSUM"))
            xpp = ctx.enter_context(tc.tile_pool(name="xpp", bufs=1, space="PSUM"))
            psing = ctx.enter_context(
                tc.tile_pool(name="psing", bufs=1, space="PSUM")
            )

            # ---- streaming index-derived inputs -----------------------------
            strm_t = {}

            def fetch_stream(i):
                s = strmq.tile([P, 4, BS], FP8, tag="strm")
                nc.sync.dma_start(s[:], strm_d[i])
                strm_t[i] = s

            for _i in range(4):
                fetch_stream(_i)

            # ---- resident SBUF tensors -------------------------------------
            te2 = sing.tile([P, 2, E], FP8, tag="te")
            nc.sync.dma_start(te2[:], te_d.rearrange("(ko p) m -> p ko m", p=P))
            wiht = sing.tile([P, 2, 4 * E], FP8, tag="wiht")
            nc.sync.dma_start(wiht[:], wiht_d.rearrange("(ko p) m -> p ko m", p=P))
            bgc = sing.tile([P, 8], F32, tag="bgc")
            nc.sync.dma_start(bgc[:], bgc_d)
            whht = sing.tile([P, 2, 4 * E], FP8, tag="whht")
            nc.sync.dma_start(whht[:], whht_d.rearrange("(ko p) m -> p ko m", p=P))
            w1t = sing.tile([P, 2, 2 * E], FP8, tag="w1t")
            nc.sync.dma_start(w1t[:], w1t_d.rearrange("(ko p) m -> p ko m", p=P))
            w2t = sing.tile([P, 4, NCL], FP8, tag="w2t")
            nc.sync.dma_start(w2t[:], w2t_d.rearrange("(ko p) m -> p ko m", p=P))
            b1c = sing.tile([P, 4], F32, tag="b1c")
            nc.sync.dma_start(b1c[:], b1c_d)
            b2c = sing.tile([P, 2], F32, tag="b2c")
            nc.sync.dma_start(b2c[:], b2c_d)
            slide4 = sing.tile([P, 2, 2 * D], FP8, tag="slide4")
            nc.sync.dma_start(slide4[:], slide4_d)
            slide5 = sing.tile([P, 2, 2 * D], FP8E5, tag="slide5")
            nc.sync.dma_start(slide5[:], slide5_d)
            ones64 = sing.tile([D, 1], F32, tag="ones64")
            nc.sync.dma_start(ones64[:], ones64_d)

            h_sb = sing.tile([P, 2, BS], FP8, tag="h")
            c_sb = sing.tile([P, 2, BS], BF16, tag="c")
            lbuf = sing.tile([P, 2, 2 * SUB, BS], BF16, tag="lbuf")
            esum_ps = psing.tile([D, BS], F32, tag="esum")
            pick_ps = psing.tile([D, BS], F32, tag="pick")

            def gate(j, x_sb, with_h):
                """Pre-act matmuls + activation for gate-dim block j (of 8)."""
                g_ps = pp.tile([P, BS], F32, tag="ps")
                nc.tensor.matmul(
                    g_ps, wiht[:, :, ts(j, P)], x_sb, start=True, stop=not with_h,
                    perf_mode=DR,
                )
                if with_h:
                    nc.tensor.matmul(
                        g_ps, whht[:, :, ts(j, P)], h_sb, start=False, stop=True,
                        perf_mode=DR,
                    )
                g_sb = gpool.tile([P, BS], BF16, tag="gate")
                nc.scalar.activation(g_sb, g_ps, GATE_FUNCS[j], bias=bgc[:, j : j + 1])
                return g_sb

            def cell_k(gt, k):
                """c[:,k] = f*c + i*g; h[:,k] = o*tanh(c). gt indexed by block j.

                i*g runs on GPSIMD in parallel with f*c on DVE; the rest of the
                recurrence-critical chain (add, tanh, h mul) stays on DVE/ACT.
                """
                ig = tpool.tile([P, BS], BF16, tag="tmp")
                nc.gpsimd.tensor_mul(ig, gt[0 + k], gt[4 + k])
                fc = tpool.tile([P, BS], BF16, tag="tmp")
                nc.vector.tensor_mul(fc, gt[2 + k], c_sb[:, k, :])
                nc.vector.tensor_add(c_sb[:, k, :], ig, fc)
                tcl = tpool.tile([P, BS], BF16, tag="tc")
                nc.scalar.activation(tcl, c_sb[:, k, :], TANH)
                nc.vector.tensor_mul(h_sb[:, k, :], gt[6 + k], tcl)

            pending_esum = []  # FIFO of deferred esum matmuls

            def defer_block(j0):
                """exp for steps j0..j0+SUB-1: one big ACTIVATE per k-tile."""
                blk = (j0 // SUB) % 2
                e = epool.tile([P, SUB, 2, BS], FP8E5, tag="e")
                for k in range(2):
                    nc.scalar.activation(
                        e[:, :, k, :],
                        lbuf[:, k, blk * SUB : (blk + 1) * SUB, :],
                        AF.Exp,
                        bias=b2c[:, k : k + 1],
                    )
                for j in range(j0, j0 + SUB):
                    pending_esum.append((j, e))

            def pop_esum(n):
                for _ in range(min(n, len(pending_esum))):
                    j, e = pending_esum.pop(0)
                    nc.tensor.matmul(
                        esum_ps,
                        slide5[:, :, D - 1 - j : 2 * D - 1 - j],
                        e[:, j % SUB, :, :],
                        start=(j == 0),
                        stop=(j == n_steps - 1),
                        perf_mode=DR,
                        skip_group_check=True,
                    )

            def gather_x(i):
                """x_i = token_embed[s_i] + pe_rows, fp8 [P, 2, BS] in SBUF."""
                strm = strm_t[i]
                x_ps = xpp.tile([P, 2, BS], F32, tag="xps")
                for t in range(2):
                    nc.tensor.matmul(
                        x_ps[:, t, :], te2[:, :, ts(t, P)], strm[:, 0:2, :],
                        start=True, stop=True, perf_mode=DR,
                    )
                x_sb = xpool.tile([P, 2, BS], FP8, tag="x")
                nc.vector.tensor_add(x_sb, x_ps, strm[:, 2:4, :])
                return x_sb

            # ---- init: h,c from lstm(pe[:,0]) with zero state ---------------
            # x0 is exactly the pex half of stream 0 (fp8, DoubleRow-packed).
            gt0 = [None] * 8
            for j in (0, 4, 2, 6, 1, 5, 3, 7):
                gt0[j] = gate(j, strm_t[0][:, 2:4, :], with_h=False)
            for k in range(2):
                nc.vector.tensor_mul(c_sb[:, k, :], gt0[0 + k], gt0[4 + k])
                tcl = tpool.tile([P, BS], BF16, tag="tc")
                nc.scalar.activation(tcl, c_sb[:, k, :], TANH)
                nc.vector.tensor_mul(h_sb[:, k, :], gt0[6 + k], tcl)

            x_t = {0: gather_x(0)}
            prev_prod = None  # (step, prod pair tile) awaiting the pick matmul

            # ---- scan ------------------------------------------------------
            # Per-iteration order: h-independent PE work first (x gather for
            # i+1, previous pick, one esum drain), then hid/gates/logits (all
            # reading h_i), then the cell update writes h_{i+1} last.
            for i in range(n_steps):
                if i + 1 < n_steps:
                    x_t[i + 1] = gather_x(i + 1)
                if prev_prod is not None:
                    pj, pprod = prev_prod
                    nc.tensor.matmul(
                        pick_ps,
                        slide4[:, :, D - 1 - pj : 2 * D - 1 - pj],
                        pprod,
                        start=(pj == 0),
                        stop=False,
                        perf_mode=DR,
                        skip_group_check=True,
                    )
                    prev_prod = None
                pop_esum(1)
                if i + 4 < n_steps:
                    fetch_stream(i + 4)

                # hid matmuls (read h_i) with relu split around the cell:
                # t0/t1 convert now, t2/t3 after the cell chain so the DVE
                # queue runs the recurrence-critical ops first.
                hid_sb = hidp.tile([P, 4, BS], FP8, tag="hid")
                hps = []
                for t in range(4):
                    hp = pp.tile([P, BS], F32, tag="ps")
                    nc.tensor.matmul(
                        hp, w1t[:, :, ts(t, P)], h_sb, start=True, stop=True,
                        perf_mode=DR,
                    )
                    hps.append(hp)
                    if t < 2:
                        nc.vector.tensor_scalar(
                            hid_sb[:, t, :], hp, b1c[:, t : t + 1], 0.0,
                            OP.add, OP.max,
                        )

                # gates k0 (i,g,f,o), the k0 cell chain while the k1 gate
                # matmuls/activations run, then k1 and its cell chain. The
                # per-half h writes come after every reader of h_i.
                gt = [None] * 8
                xi = x_t.pop(i)
                for j in (0, 4, 2, 6):
                    gt[j] = gate(j, xi, with_h=True)
                cell_k(gt, 0)
                for j in (1, 5, 3, 7):
                    gt[j] = gate(j, xi, with_h=True)
                cell_k(gt, 1)

                for t in (2, 3):
                    nc.vector.tensor_scalar(
                        hid_sb[:, t, :], hps[t], b1c[:, t : t + 1], 0.0,
                        OP.add, OP.max,
                    )

                # logits = W2 @ hid: 2 out blocks x 2 DoubleRow each
                l_ps = []
                for t in range(2):
                    lp = pp.tile([P, BS], F32, tag="ps")
                    nc.tensor.matmul(
                        lp, w2t[:, 0:2, ts(t, P)], hid_sb[:, 0:2, :], start=True,
                        stop=False, perf_mode=DR,
                    )
                    nc.tensor.matmul(
                        lp, w2t[:, 2:4, ts(t, P)], hid_sb[:, 2:4, :], start=False,
                        stop=True, perf_mode=DR,
                    )
                    l_ps.append(lp)

                # stash logits (frees the PSUM bank) + picked-logit product
                slot = i % (2 * SUB)
                strm_i = strm_t.pop(i)
                prods = prodp.tile([P, 2, BS], FP8, tag="prod")
                for k in range(2):
                    nc.vector.tensor_copy(out=lbuf[:, k, slot, :], in_=l_ps[k])
                    nc.vector.scalar_tensor_tensor(
                        prods[:, k, :], l_ps[k], b2c[:, k : k + 1],
                        strm_i[:, k, :], OP.add, OP.mult,
                    )
                prev_prod = (i, prods)

                if (i + 1) % SUB == 0 and i + 1 >= SUB and i + 1 < n_steps:
                    defer_block(i + 1 - SUB)

            # final pick + last deferred block + drain
            pj, pprod = prev_prod
            nc.tensor.matmul(
                pick_ps,
                slide4[:, :, D - 1 - pj : 2 * D - 1 - pj],
                pprod,
                start=False,
                stop=True,
                perf_mode=DR,
                skip_group_check=True,
            )
            defer_block(n_steps - SUB)
            pop_esum(len(pending_esum))

            # ---- epilogue: out = sum_j (pick_j - ln(esum_j)) ----------------
            ln_e = sing.tile([D, BS], F32, tag="lne")
            nc.scalar.activation(ln_e, esum_ps, AF.Ln)
            diff = sing.tile([D, BS], F32, tag="diff")
            nc.vector.tensor_sub(diff, pick_ps, ln_e)
            fin_ps = pp.tile([1, BS], F32, tag="ps")
            nc.tensor.matmul(fin_ps, ones64[:, 0:1], diff, start=True, stop=True)
            out_sb = sing.tile([1, BS], F32, tag="outsb")
            nc.scalar.activation(out_sb, fin_ps, AF.Copy)
            nc.sync.dma_start(out_d, out_sb)

    nc.compile()
    return nc


def prep_inputs(token_embed, W_ih, b_ih, W_hh, b_hh, W1, b1, W2, b2, pos_list,
                input_samples):
    """Host-side layout prep -> per-core in_maps for run_bass_kernel_spmd."""
    f = np.float32
    slide = np.zeros((P, 2, 2 * D), f)
    slide[:, :, D - 1] = 1.0
    shared = {
        "wiht": np.ascontiguousarray(np.asarray(W_ih, f).T).astype(NPE4),
        "whht": np.ascontiguousarray(np.asarray(W_hh, f).T).astype(NPE4),
        "w1t": np.ascontiguousarray(np.asarray(W1, f).T).astype(NPE4),
        "w2t": np.ascontiguousarray(np.asarray(W2, f).T).astype(NPE4),
        "te": np.asarray(token_embed, f).astype(NPE4),
        "bgc": np.ascontiguousarray(
            (np.asarray(b_ih, f) + np.asarray(b_hh, f)).reshape(8, P).T
        ),
        "b1c": np.ascontiguousarray(np.asarray(b1, f).reshape(4, P).T),
        "b2c": np.ascontiguousarray(np.asarray(b2, f).reshape(2, P).T),
        "slide4": slide.astype(NPE4),
        "slide5": slide.astype(NPE5),
        "ones64": np.ones((D, 1), f),
    }
    pe_e4 = _pe_table().astype(NPE4)  # quantize the table once, then gather
    samples = np.asarray(input_samples)
    poss = np.asarray(pos_list)
    didx = np.arange(D)[:, None]
    bidx = np.arange(BS)[None, :]
    in_maps = []
    for c in range(NCORES):
        lo, hi = c * BS, (c + 1) * BS
        sT = np.ascontiguousarray(samples[lo:hi].T)  # [D, BS]
        pT = np.ascontiguousarray(poss[lo:hi].T)
        strm = np.zeros((D, P, 4, BS), NPE4)
        strm[didx, sT % P, sT // P, bidx] = 1.0  # one-hot blocks 0:2
        # pex blocks 2:4: strm[i, p, 2+k, b] = pe_table[pos[b, i], 128k + p]
        strm[:, :, 2:4, :] = pe_e4[pT].reshape(D, BS, 2, P).transpose(0, 3, 2, 1)
        m = dict(shared)
        m["strm"] = strm
        in_maps.append(m)
    return in_maps


_CACHE = {}


def kernel(**inputs) -> np.ndarray:
    if "nc" not in _CACHE:
        _CACHE["nc"] = build_bass()
    nc = _CACHE["nc"]
    in_maps = prep_inputs(**inputs)
    res = bass_utils.run_bass_kernel_spmd(nc, in_maps, core_ids=list(range(NCORES)))
    _CACHE["last_results"] = res
    out = np.empty((B, 1), np.float32)
    for c in range(NCORES):
        out[c * BS : (c + 1) * BS, 0] = np.asarray(
            res.results[c]["out"], np.float32
        ).reshape(BS)
    return out


# revision 16
# speedup vs baseline: 1.1805x; 1.0169x over previous
"""Trainium2 Bass kernel for nn_CondRnnSampler.

Computes, for each batch row b:
    out[b] = sum_i log_softmax(MLP(h_i))[s_i]  over a 64-step LSTM scan,
with the LSTM consuming x_i = token_embed[s_i] + pos_enc(p_i).

Strategy: pure data parallel over 8 NeuronCores (512 batch rows each).
All activations are feature-major ([features-on-partitions, batch-on-free]) so
every layer is a stationary-weight matmul with N=512 moving columns. All large
matmuls run in fp8e4 with MatmulPerfMode.DoubleRow: both 128-row k-tiles of the
contraction are packed into one instruction ([K=128, 2, M] stationary against
[K=128, 2, N] moving), halving the PE instruction count vs bf16. Accumulation
stays fp32 in PSUM; the LSTM cell state c is bf16.

Per step one merged DMA delivers the host-built index encodings: the sample
one-hot (fp8, DoubleRow-packed) and the positional-encoding rows pe_table[p]
(fp8, same packing) in a single [128, 4, 512] tile. token_embed[s] is a
one-hot DoubleRow matmul; x = (te gather PSUM) + pex in one fused DVE add.
The per-step picked-logit and softmax-denominator reductions are single
DoubleRow matmuls against a sliding selector so step j lands on PSUM partition
j; exp is deferred into SUB-step blocks (one big ACTIVATE per k-tile, output
fp8e5 for range) so the ACT table set switches only twice per SUB steps.

Step order follows the proven baseline: all consumers of h_i (x-gather for
i+1, previous pick, esum drain, hid, gates, logits) are emitted before the
cell update writes h_{i+1}, keeping the recurrence chain short and the
h-independent work in front of it.
"""

import sys

sys.path.insert(0, "/opt/trn_rl_repo")

from contextlib import ExitStack

import ml_dtypes
import numpy as np

import concourse.bacc as bacc
import concourse.tile as tile
from concourse import bass_utils, mybir
from concourse.bass import ts

B, D, E, NCL = 4096, 64, 256, 256  # batch, steps, embed, n_choices
NCORES = 8
BS = B // NCORES  # 512 rows per core
P = 128
SUB = 8  # deferred-softmax block (steps)

AF = mybir.ActivationFunctionType
OP = mybir.AluOpType
F32 = mybir.dt.float32
BF16 = mybir.dt.bfloat16
FP8 = mybir.dt.float8e4
FP8E5 = mybir.dt.float8e5
DR = mybir.MatmulPerfMode.DoubleRow

NPBF = ml_dtypes.bfloat16
NPE4 = ml_dtypes.float8_e4m3
NPE5 = ml_dtypes.float8_e5m2

SIG = AF.Sigmoid
TANH = AF.Tanh
# gate-dim blocks j of 128 over 4E=1024: (i0,i1,f0,f1,g0,g1,o0,o1)
GATE_FUNCS = [SIG, SIG, SIG, SIG, TANH, TANH, SIG, SIG]


def _pe_table() -> np.ndarray:
    half = np.float32(E // 2)
    inv = (
        np.float32(1.0)
        / (np.float32(10000.0) ** (np.arange(E // 2, dtype=np.float32) / half))
    ).astype(np.float32)
    pos = np.arange(D, dtype=np.float32)[:, None]
    ang = pos * inv[None, :]
    return np.concatenate([np.sin(ang), np.cos(ang)], axis=1).astype(np.float32)


def build_bass(n_steps: int = D):
    """Build the per-core Bass program (identical on all 8 cores)."""
    nc = bacc.Bacc("TRN2", debug=False, target_bir_lowering=False, num_devices=NCORES)

    def din(name, shape, dt=FP8):
        return nc.dram_tensor(name, list(shape), dt, kind="ExternalInput").ap()

    wiht_d = din("wiht", (E, 4 * E))  # W_ih.T
    whht_d = din("whht", (E, 4 * E))  # W_hh.T
    w1t_d = din("w1t", (E, 2 * E))  # W1.T
    w2t_d = din("w2t", (2 * E, NCL))  # W2.T
    te_d = din("te", (NCL, E))  # token_embed (lhsT for the gather)
    bgc_d = din("bgc", (P, 8), F32)
    b1c_d = din("b1c", (P, 4), F32)
    b2c_d = din("b2c", (P, 2), F32)
    # per-step stream: blocks (oh_k0, oh_k1, pex_k0, pex_k1)
    strm_d = din("strm", (D, P, 4, BS))
    slide4_d = din("slide4", (P, 2, 2 * D))  # pick selector (fp8e4)
    slide5_d = din("slide5", (P, 2, 2 * D), FP8E5)  # esum selector
    ones64_d = din("ones64", (D, 1), F32)
    out_d = nc.dram_tensor("out", [1, BS], F32, kind="ExternalOutput").ap()

    with tile.TileContext(nc) as tc:
        with ExitStack() as ctx:
            sing = ctx.enter_context(tc.tile_pool(name="sing", bufs=1))
            strmq = ctx.enter_context(tc.tile_pool(name="strmq", bufs=6))
            xpool = ctx.enter_context(tc.tile_pool(name="xpool", bufs=3))
            gpool = ctx.enter_context(tc.tile_pool(name="gpool", bufs=10))
            tpool = ctx.enter_context(tc.tile_pool(name="tpool", bufs=4))
            hidp = ctx.enter_context(tc.tile_pool(name="hidp", bufs=2))
            prodp = ctx.enter_context(tc.tile_pool(name="prodp", bufs=3))
            epool = ctx.enter_context(tc.tile_pool(name="epool", bufs=2))
            pp = ctx.enter_context(tc.tile_pool(name="pp", bufs=4, space="PSUM"))
            xpp = ctx.enter_context(tc.tile_pool(name="xpp", bufs=1, space="PSUM"))
            psing = ctx.enter_context(
                tc.tile_pool(name="psing", bufs=1, space="PSUM")
            )

            # ---- streaming index-derived inputs -----------------------------
            strm_t = {}

            def fetch_stream(i):
                s = strmq.tile([P, 4, BS], FP8, tag="strm")
                nc.sync.dma_start(s[:], strm_d[i])
                strm_t[i] = s

            for _i in range(4):
                fetch_stream(_i)

            # ---- resident SBUF tensors -------------------------------------
            te2 = sing.tile([P, 2, E], FP8, tag="te")
            nc.sync.dma_start(te2[:], te_d.rearrange("(ko p) m -> p ko m", p=P))
            wiht = sing.tile([P, 2, 4 * E], FP8, tag="wiht")
            nc.sync.dma_start(wiht[:], wiht_d.rearrange("(ko p) m -> p ko m", p=P))
            bgc = sing.tile([P, 8], F32, tag="bgc")
            nc.sync.dma_start(bgc[:], bgc_d)
            whht = sing.tile([P, 2, 4 * E], FP8, tag="whht")
            nc.sync.dma_start(whht[:], whht_d.rearrange("(ko p) m -> p ko m", p=P))
            w1t = sing.tile([P, 2, 2 * E], FP8, tag="w1t")
            nc.sync.dma_start(w1t[:], w1t_d.rearrange("(ko p) m -> p ko m", p=P))
            w2t = sing.tile([P, 4, NCL], FP8, tag="w2t")
            nc.sync.dma_start(w2t[:], w2t_d.rearrange("(ko p) m -> p ko m", p=P))
            b1c = sing.tile([P, 4], F32, tag="b1c")
            nc.sync.dma_start(b1c[:], b1c_d)
            b2c = sing.tile([P, 2], F32, tag="b2c")
            nc.sync.dma_start(b2c[:], b2c_d)
            slide4 = sing.tile([P, 2, 2 * D], FP8, tag="slide4")
            nc.sync.dma_start(slide4[:], slide4_d)
            slide5 = sing.tile([P, 2, 2 * D], FP8E5, tag="slide5")
            nc.sync.dma_start(slide5[:], slide5_d)
            ones64 = sing.tile([D, 1], F32, tag="ones64")
            nc.sync.dma_start(ones64[:], ones64_d)

            h_sb = sing.tile([P, 2, BS], FP8, tag="h")
            c_sb = sing.tile([P, 2, BS], BF16, tag="c")
            lbuf = sing.tile([P, 2, 2 * SUB, BS], BF16, tag="lbuf")
            esum_ps = psing.tile([D, BS], F32, tag="esum")
            pick_ps = psing.tile([D, BS], F32, tag="pick")

            def gate(j, x_sb, with_h):
                """Pre-act matmuls + activation for gate-dim block j (of 8)."""
                g_ps = pp.tile([P, BS], F32, tag="ps")
                nc.tensor.matmul(
                    g_ps, wiht[:, :, ts(j, P)], x_sb, start=True, stop=not with_h,
                    perf_mode=DR,
                )
                if with_h:
                    nc.tensor.matmul(
                        g_ps, whht[:, :, ts(j, P)], h_sb, start=False, stop=True,
                        perf_mode=DR,
                    )
                g_sb = gpool.tile([P, BS], BF16, tag="gate")
                nc.scalar.activation(g_sb, g_ps, GATE_FUNCS[j], bias=bgc[:, j : j + 1])
                return g_sb

            def cell_k(gt, k):
                """c[:,k] = f*c + i*g; h[:,k] = o*tanh(c). gt indexed by block j.

                i*g runs on GPSIMD in parallel with f*c on DVE; the rest of the
                recurrence-critical chain (add, tanh, h mul) stays on DVE/ACT.
                """
                ig = tpool.tile([P, BS], BF16, tag="tmp")
                nc.gpsimd.tensor_mul(ig, gt[0 + k], gt[4 + k])
                fc = tpool.tile([P, BS], BF16, tag="tmp")
                nc.vector.tensor_mul(fc, gt[2 + k], c_sb[:, k, :])
                nc.vector.tensor_add(c_sb[:, k, :], ig, fc)
                tcl = tpool.tile([P, BS], BF16, tag="tc")
                nc.scalar.activation(tcl, c_sb[:, k, :], TANH)
                nc.vector.tensor_mul(h_sb[:, k, :], gt[6 + k], tcl)

            pending_esum = []  # FIFO of deferred esum matmuls

            def defer_block(j0):
                """exp for steps j0..j0+SUB-1: one big ACTIVATE per k-tile."""
                blk = (j0 // SUB) % 2
                e = epool.tile([P, SUB, 2, BS], FP8E5, tag="e")
                for k in range(2):
                    nc.scalar.activation(
                        e[:, :, k, :],
                        lbuf[:, k, blk * SUB : (blk + 1) * SUB, :],
                        AF.Exp,
                        bias=b2c[:, k : k + 1],
                    )
                for j in range(j0, j0 + SUB):
                    pending_esum.append((j, e))

            def pop_esum(n):
                for _ in range(min(n, len(pending_esum))):
                    j, e = pending_esum.pop(0)
                    nc.tensor.matmul(
                        esum_ps,
                        slide5[:, :, D - 1 - j : 2 * D - 1 - j],
                        e[:, j % SUB, :, :],
                        start=(j == 0),
                        stop=(j == n_steps - 1),
                        perf_mode=DR,
                        skip_group_check=True,
                    )

            def gather_x(i):
                """x_i = token_embed[s_i] + pe_rows, fp8 [P, 2, BS] in SBUF."""
                strm = strm_t[i]
                x_ps = xpp.tile([P, 2, BS], F32, tag="xps")
                for t in range(2):
                    nc.tensor.matmul(
                        x_ps[:, t, :], te2[:, :, ts(t, P)], strm[:, 0:2, :],
                        start=True, stop=True, perf_mode=DR,
                    )
                x_sb = xpool.tile([P, 2, BS], FP8, tag="x")
                nc.vector.tensor_add(x_sb, x_ps, strm[:, 2:4, :])
                return x_sb

            # ---- init: h,c from lstm(pe[:,0]) with zero state ---------------
            # x0 is exactly the pex half of stream 0 (fp8, DoubleRow-packed).
            gt0 = [None] * 8
            for j in (0, 4, 2, 6, 1, 5, 3, 7):
                gt0[j] = gate(j, strm_t[0][:, 2:4, :], with_h=False)
            for k in range(2):
                nc.vector.tensor_mul(c_sb[:, k, :], gt0[0 + k], gt0[4 + k])
                tcl = tpool.tile([P, BS], BF16, tag="tc")
                nc.scalar.activation(tcl, c_sb[:, k, :], TANH)
                nc.vector.tensor_mul(h_sb[:, k, :], gt0[6 + k], tcl)

            x_t = {0: gather_x(0)}
            prev_prod = None  # (step, prod pair tile) awaiting the pick matmul

            # ---- scan ------------------------------------------------------
            # Per-iteration order: h-independent PE work first (x gather for
            # i+1, previous pick, one esum drain), then hid/gates/logits (all
            # reading h_i), then the cell update writes h_{i+1} last.
            for i in range(n_steps):
                if i + 1 < n_steps:
                    x_t[i + 1] = gather_x(i + 1)
                if prev_prod is not None:
                    pj, pprod = prev_prod
                    nc.tensor.matmul(
                        pick_ps,
                        slide4[:, :, D - 1 - pj : 2 * D - 1 - pj],
                        pprod,
                        start=(pj == 0),
                        stop=False,
                        perf_mode=DR,
                        skip_group_check=True,
                    )
                    prev_prod = None
                pop_esum(1)
                if i + 4 < n_steps:
                    fetch_stream(i + 4)

                # hid = relu(W1 @ h + b1): 4 blocks (relu on DVE; PSUM source)
                hid_sb = hidp.tile([P, 4, BS], FP8, tag="hid")
                for t in range(4):
                    hp = pp.tile([P, BS], F32, tag="ps")
                    nc.tensor.matmul(
                        hp, w1t[:, :, ts(t, P)], h_sb, start=True, stop=True,
                        perf_mode=DR,
                    )
                    nc.vector.tensor_scalar(
                        hid_sb[:, t, :], hp, b1c[:, t : t + 1], 0.0, OP.add, OP.max
                    )

                # gates for step i: k0 blocks (i,g,f,o) then k1
                gt = [None] * 8
                xi = x_t.pop(i)
                for j in (0, 4, 2, 6, 1, 5, 3, 7):
                    gt[j] = gate(j, xi, with_h=True)

                # logits = W2 @ hid: 2 out blocks x 2 DoubleRow each
                l_ps = []
                for t in range(2):
                    lp = pp.tile([P, BS], F32, tag="ps")
                    nc.tensor.matmul(
                        lp, w2t[:, 0:2, ts(t, P)], hid_sb[:, 0:2, :], start=True,
                        stop=False, perf_mode=DR,
                    )
                    nc.tensor.matmul(
                        lp, w2t[:, 2:4, ts(t, P)], hid_sb[:, 2:4, :], start=False,
                        stop=True, perf_mode=DR,
                    )
                    l_ps.append(lp)

                # cell update (writes h_{i+1}; everything above read h_i)
                cell_k(gt, 0)
                cell_k(gt, 1)

                # stash logits (frees the PSUM bank) + picked-logit product
                slot = i % (2 * SUB)
                strm_i = strm_t.pop(i)
                prods = prodp.tile([P, 2, BS], FP8, tag="prod")
                for k in range(2):
                    nc.vector.tensor_copy(out=lbuf[:, k, slot, :], in_=l_ps[k])
                    nc.vector.scalar_tensor_tensor(
                        prods[:, k, :], l_ps[k], b2c[:, k : k + 1],
                        strm_i[:, k, :], OP.add, OP.mult,
                    )
                prev_prod = (i, prods)

                if (i + 1) % SUB == 0 and i + 1 >= SUB and i + 1 < n_steps:
                    defer_block(i + 1 - SUB)

            # final pick + last deferred block + drain
            pj, pprod = prev_prod
            nc.tensor.matmul(
                pick_ps,
                slide4[:, :, D - 1 - pj : 2 * D - 1 - pj],
                pprod,
                start=False,
                stop=True,
                perf_mode=DR,
                skip_group_check=True,
            )
            defer_block(n_steps - SUB)
            pop_esum(len(pending_esum))

            # ---- epilogue: out = sum_j (pick_j - ln(esum_j)) ----------------
            ln_e = sing.tile([D, BS], F32, tag="lne")
            nc.scalar.activation(ln_e, esum_ps, AF.Ln)
            diff = sing.tile([D, BS], F32, tag="diff")
            nc.vector.tensor_sub(diff, pick_ps, ln_e)
            fin_ps = pp.tile([1, BS], F32, tag="ps")
            nc.tensor.matmul(fin_ps, ones64[:, 0:1], diff, start=True, stop=True)
            out_sb = sing.tile([1, BS], F32, tag="outsb")
            nc.scalar.activation(out_sb, fin_ps, AF.Copy)
            nc.sync.dma_start(out_d, out_sb)

    nc.compile()
    return nc


def prep_inputs(token_embed, W_ih, b_ih, W_hh, b_hh, W1, b1, W2, b2, pos_list,
                input_samples):
    """Host-side layout prep -> per-core in_maps for run_bass_kernel_spmd."""
    f = np.float32
    slide = np.zeros((P, 2, 2 * D), f)
    slide[:, :, D - 1] = 1.0
    shared = {
        "wiht": np.ascontiguousarray(np.asarray(W_ih, f).T).astype(NPE4),
        "whht": np.ascontiguousarray(np.asarray(W_hh, f).T).astype(NPE4),
        "w1t": np.ascontiguousarray(np.asarray(W1, f).T).astype(NPE4),
        "w2t": np.ascontiguousarray(np.asarray(W2, f).T).astype(NPE4),
        "te": np.asarray(token_embed, f).astype(NPE4),
        "bgc": np.ascontiguousarray(
            (np.asarray(b_ih, f) + np.asarray(b_hh, f)).reshape(8, P).T
        ),
        "b1c": np.ascontiguousarray(np.asarray(b1, f).reshape(4, P).T),
        "b2c": np.ascontiguousarray(np.asarray(b2, f).reshape(2, P).T),
        "slide4": slide.astype(NPE4),
        "slide5": slide.astype(NPE5),
        "ones64": np.ones((D, 1), f),
    }
    pe_e4 = _pe_table().astype(NPE4)  # quantize the table once, then gather
    samples = np.asarray(input_samples)
    poss = np.asarray(pos_list)
    didx = np.arange(D)[:, None]
    bidx = np.arange(BS)[None, :]
    in_maps = []
    for c in range(NCORES):
        lo, hi = c * BS, (c + 1) * BS
        sT = np.ascontiguousarray(samples[lo:hi].T)  # [D, BS]
        pT = np.ascontiguousarray(poss[lo:hi].T)
        strm = np.zeros((D, P, 4, BS), NPE4)
        strm[didx, sT % P, sT // P, bidx] = 1.0  # one-hot blocks 0:2
        # pex blocks 2:4: strm[i, p, 2+k, b] = pe_table[pos[b, i], 128k + p]
        strm[:, :, 2:4, :] = pe_e4[pT].reshape(D, BS, 2, P).transpose(0, 3, 2, 1)
        m = dict(shared)
        m["strm"] = strm
        in_maps.append(m)
    return in_maps


_CACHE = {}


def kernel(**inputs) -> np.ndarray:
    if "nc" not in _CACHE:
        _CACHE["nc"] = build_bass()
    nc = _CACHE["nc"]
    in_maps = prep_inputs(**inputs)
    res = bass_utils.run_bass_kernel_spmd(nc, in_maps, core_ids=list(range(NCORES)))
    _CACHE["last_results"] = res
    out = np.empty((B, 1), np.float32)
    for c in range(NCORES):
        out[c * BS : (c + 1) * BS, 0] = np.asarray(
            res.results[c]["out"], np.float32
        ).reshape(BS)
    return out


# revision 19
# speedup vs baseline: 1.4914x; 1.2634x over previous
"""Trainium2 Bass kernel for nn_CondRnnSampler.

Computes, for each batch row b:
    out[b] = sum_i log_softmax(MLP(h_i))[s_i]  over a 64-step LSTM scan,
with the LSTM consuming x_i = token_embed[s_i] + pos_enc(p_i).

Strategy: pure data parallel over 8 NeuronCores (512 batch rows each).
All activations are feature-major ([features-on-partitions, batch-on-free]) so
every layer is a stationary-weight matmul with N=512 moving columns. All large
matmuls run in fp8e4 with MatmulPerfMode.DoubleRow: both 128-row k-tiles of the
contraction are packed into one instruction ([K=128, 2, M] stationary against
[K=128, 2, N] moving), halving the PE instruction count vs bf16. Accumulation
stays fp32 in PSUM; the LSTM cell state c is bf16.

Per step one merged DMA delivers the host-built index encodings: the sample
one-hot (fp8, DoubleRow-packed) and the positional-encoding rows pe_table[p]
(fp8, same packing) in a single [128, 4, 512] tile. token_embed[s] is a
one-hot DoubleRow matmul; x = (te gather PSUM) + pex in one fused DVE add.
The per-step picked-logit and softmax-denominator reductions are single
DoubleRow matmuls against a sliding selector so step j lands on PSUM partition
j; exp is deferred into SUB-step blocks (one big ACTIVATE per k-tile, output
fp8e5 for range) so the ACT table set switches only twice per SUB steps.

Step order follows the proven baseline: all consumers of h_i (x-gather for
i+1, previous pick, esum drain, hid, gates, logits) are emitted before the
cell update writes h_{i+1}, keeping the recurrence chain short and the
h-independent work in front of it.
"""

import sys

sys.path.insert(0, "/opt/trn_rl_repo")

from contextlib import ExitStack

import ml_dtypes
import numpy as np

import concourse.bacc as bacc
import concourse.tile as tile
from concourse import bass_utils, mybir
from concourse.bass import ts

B, D, E, NCL = 4096, 64, 256, 256  # batch, steps, embed, n_choices
NCORES = 8
BS = B // NCORES  # 512 rows per core
P = 128
SUB = 8  # deferred-softmax block (steps)

AF = mybir.ActivationFunctionType
OP = mybir.AluOpType
F32 = mybir.dt.float32
BF16 = mybir.dt.bfloat16
FP8 = mybir.dt.float8e4
FP8E5 = mybir.dt.float8e5
DR = mybir.MatmulPerfMode.DoubleRow

NPBF = ml_dtypes.bfloat16
NPE4 = ml_dtypes.float8_e4m3
NPE5 = ml_dtypes.float8_e5m2

SIG = AF.Sigmoid
TANH = AF.Tanh
# gate-dim blocks j of 128 over 4E=1024: (i0,i1,f0,f1,g0,g1,o0,o1)
GATE_FUNCS = [SIG, SIG, SIG, SIG, TANH, TANH, SIG, SIG]


def _pe_table() -> np.ndarray:
    half = np.float32(E // 2)
    inv = (
        np.float32(1.0)
        / (np.float32(10000.0) ** (np.arange(E // 2, dtype=np.float32) / half))
    ).astype(np.float32)
    pos = np.arange(D, dtype=np.float32)[:, None]
    ang = pos * inv[None, :]
    return np.concatenate([np.sin(ang), np.cos(ang)], axis=1).astype(np.float32)


def build_bass(n_steps: int = D):
    """Build the per-core Bass program (identical on all 8 cores)."""
    nc = bacc.Bacc("TRN2", debug=False, target_bir_lowering=False, num_devices=NCORES)

    def din(name, shape, dt=FP8):
        return nc.dram_tensor(name, list(shape), dt, kind="ExternalInput").ap()

    wiht_d = din("wiht", (E, 4 * E))  # W_ih.T
    whht_d = din("whht", (E, 4 * E))  # W_hh.T
    w1t_d = din("w1t", (E, 2 * E))  # W1.T
    w2t_d = din("w2t", (2 * E, NCL))  # W2.T
    te_d = din("te", (NCL, E))  # token_embed (lhsT for the gather)
    bgc_d = din("bgc", (P, 8), F32)
    b1c_d = din("b1c", (P, 4), F32)
    b2c_d = din("b2c", (P, 2), F32)
    # per-step stream: blocks (oh_k0, oh_k1, pex_k0, pex_k1)
    strm_d = din("strm", (D, P, 4, BS))
    slide4_d = din("slide4", (P, 2, 2 * D))  # pick selector (fp8e4)
    slide5_d = din("slide5", (P, 2, 2 * D), FP8E5)  # esum selector
    ones64_d = din("ones64", (D, 1), F32)
    out_d = nc.dram_tensor("out", [1, BS], F32, kind="ExternalOutput").ap()

    with tile.TileContext(nc) as tc:
        with ExitStack() as ctx:
            sing = ctx.enter_context(tc.tile_pool(name="sing", bufs=1))
            strmq = ctx.enter_context(tc.tile_pool(name="strmq", bufs=6))
            xpool = ctx.enter_context(tc.tile_pool(name="xpool", bufs=3))
            gpool = ctx.enter_context(tc.tile_pool(name="gpool", bufs=10))
            tpool = ctx.enter_context(tc.tile_pool(name="tpool", bufs=4))
            hidp = ctx.enter_context(tc.tile_pool(name="hidp", bufs=2))
            prodp = ctx.enter_context(tc.tile_pool(name="prodp", bufs=3))
            epool = ctx.enter_context(tc.tile_pool(name="epool", bufs=2))
            pp = ctx.enter_context(tc.tile_pool(name="pp", bufs=4, space="PSUM"))
            xpp = ctx.enter_context(tc.tile_pool(name="xpp", bufs=1, space="PSUM"))
            psing = ctx.enter_context(
                tc.tile_pool(name="psing", bufs=1, space="PSUM")
            )

            # ---- streaming index-derived inputs -----------------------------
            strm_t = {}

            def fetch_stream(i):
                s = strmq.tile([P, 4, BS], FP8, tag="strm")
                nc.sync.dma_start(s[:], strm_d[i])
                strm_t[i] = s

            for _i in range(4):
                fetch_stream(_i)

            # ---- resident SBUF tensors -------------------------------------
            te2 = sing.tile([P, 2, E], FP8, tag="te")
            nc.sync.dma_start(te2[:], te_d.rearrange("(ko p) m -> p ko m", p=P))
            wiht = sing.tile([P, 2, 4 * E], FP8, tag="wiht")
            nc.sync.dma_start(wiht[:], wiht_d.rearrange("(ko p) m -> p ko m", p=P))
            bgc = sing.tile([P, 8], F32, tag="bgc")
            nc.sync.dma_start(bgc[:], bgc_d)
            whht = sing.tile([P, 2, 4 * E], FP8, tag="whht")
            nc.sync.dma_start(whht[:], whht_d.rearrange("(ko p) m -> p ko m", p=P))
            w1t = sing.tile([P, 2, 2 * E], FP8, tag="w1t")
            nc.sync.dma_start(w1t[:], w1t_d.rearrange("(ko p) m -> p ko m", p=P))
            w2t = sing.tile([P, 4, NCL], FP8, tag="w2t")
            nc.sync.dma_start(w2t[:], w2t_d.rearrange("(ko p) m -> p ko m", p=P))
            b1c = sing.tile([P, 4], F32, tag="b1c")
            nc.sync.dma_start(b1c[:], b1c_d)
            b2c = sing.tile([P, 2], F32, tag="b2c")
            nc.sync.dma_start(b2c[:], b2c_d)
            slide4 = sing.tile([P, 2, 2 * D], FP8, tag="slide4")
            nc.sync.dma_start(slide4[:], slide4_d)
            slide5 = sing.tile([P, 2, 2 * D], FP8E5, tag="slide5")
            nc.sync.dma_start(slide5[:], slide5_d)
            ones64 = sing.tile([D, 1], F32, tag="ones64")
            nc.sync.dma_start(ones64[:], ones64_d)

            h0_sb = sing.tile([P, 2, BS], FP8, tag="h0")
            h1_sb = sing.tile([P, 2, BS], FP8, tag="h1")
            h_t = [h0_sb, h1_sb]
            c_sb = sing.tile([P, 2, BS], BF16, tag="c")
            lbuf = sing.tile([P, 2, 2 * SUB, BS], BF16, tag="lbuf")
            esum_ps = psing.tile([D, BS], F32, tag="esum")
            pick_ps = psing.tile([D, BS], F32, tag="pick")

            def gate(j, x_sb, hsrc, with_h):
                """Pre-act matmuls + activation for gate-dim block j (of 8)."""
                g_ps = pp.tile([P, BS], F32, tag="ps")
                nc.tensor.matmul(
                    g_ps, wiht[:, :, ts(j, P)], x_sb, start=True, stop=not with_h,
                    perf_mode=DR,
                )
                if with_h:
                    nc.tensor.matmul(
                        g_ps, whht[:, :, ts(j, P)], hsrc, start=False, stop=True,
                        perf_mode=DR,
                    )
                g_sb = gpool.tile([P, BS], BF16, tag="gate")
                nc.scalar.activation(g_sb, g_ps, GATE_FUNCS[j], bias=bgc[:, j : j + 1])
                return g_sb

            def cell_k(gt, k, hdst):
                """c[:,k] = f*c + i*g; h[:,k] = o*tanh(c). gt indexed by block j.

                i*g runs on GPSIMD in parallel with f*c on DVE; the rest of the
                recurrence-critical chain (add, tanh, h mul) stays on DVE/ACT.
                """
                ig = tpool.tile([P, BS], BF16, tag="tmp")
                nc.gpsimd.tensor_mul(ig, gt[0 + k], gt[4 + k])
                fc = tpool.tile([P, BS], BF16, tag="tmp")
                nc.vector.tensor_mul(fc, gt[2 + k], c_sb[:, k, :])
                nc.vector.tensor_add(c_sb[:, k, :], ig, fc)
                tcl = tpool.tile([P, BS], BF16, tag="tc")
                nc.scalar.activation(tcl, c_sb[:, k, :], TANH)
                nc.vector.tensor_mul(hdst[:, k, :], gt[6 + k], tcl)

            pending_esum = []  # FIFO of deferred esum matmuls

            def defer_block(j0):
                """exp for steps j0..j0+SUB-1: one big ACTIVATE per k-tile."""
                blk = (j0 // SUB) % 2
                e = epool.tile([P, SUB, 2, BS], FP8E5, tag="e")
                for k in range(2):
                    nc.scalar.activation(
                        e[:, :, k, :],
                        lbuf[:, k, blk * SUB : (blk + 1) * SUB, :],
                        AF.Exp,
                        bias=b2c[:, k : k + 1],
                    )
                for j in range(j0, j0 + SUB):
                    pending_esum.append((j, e))

            def pop_esum(n):
                for _ in range(min(n, len(pending_esum))):
                    j, e = pending_esum.pop(0)
                    nc.tensor.matmul(
                        esum_ps,
                        slide5[:, :, D - 1 - j : 2 * D - 1 - j],
                        e[:, j % SUB, :, :],
                        start=(j == 0),
                        stop=(j == n_steps - 1),
                        perf_mode=DR,
                        skip_group_check=True,
                    )

            def gather_x(i):
                """x_i = token_embed[s_i] + pe_rows, fp8 [P, 2, BS] in SBUF."""
                strm = strm_t[i]
                x_ps = xpp.tile([P, 2, BS], F32, tag="xps")
                for t in range(2):
                    nc.tensor.matmul(
                        x_ps[:, t, :], te2[:, :, ts(t, P)], strm[:, 0:2, :],
                        start=True, stop=True, perf_mode=DR,
                    )
                x_sb = xpool.tile([P, 2, BS], FP8, tag="x")
                nc.vector.tensor_add(x_sb, x_ps, strm[:, 2:4, :])
                return x_sb

            # ---- init: h,c from lstm(pe[:,0]) with zero state ---------------
            # x0 is exactly the pex half of stream 0 (fp8, DoubleRow-packed).
            gt0 = [None] * 8
            for j in (0, 4, 2, 6, 1, 5, 3, 7):
                gt0[j] = gate(j, strm_t[0][:, 2:4, :], None, with_h=False)
            for k in range(2):
                nc.vector.tensor_mul(c_sb[:, k, :], gt0[0 + k], gt0[4 + k])
                tcl = tpool.tile([P, BS], BF16, tag="tc")
                nc.scalar.activation(tcl, c_sb[:, k, :], TANH)
                nc.vector.tensor_mul(h_t[0][:, k, :], gt0[6 + k], tcl)

            x_t = {0: gather_x(0)}
            prev_prod = None  # (step, prod pair tile) awaiting the pick matmul

            def mlp_step(m):
                """hid/logits/logit-stash/pick-product for step m.

                Runs one iteration behind the recurrence: reads h_t[m % 2],
                which was finalized in the previous iteration, so every input
                is ready at iteration start and the PE/DVE queues never wait
                on the fresh h.
                """
                hsrc = h_t[m % 2]
                hid_sb = hidp.tile([P, 4, BS], FP8, tag="hid")
                for t in range(4):
                    hp = pp.tile([P, BS], F32, tag="ps")
                    nc.tensor.matmul(
                        hp, w1t[:, :, ts(t, P)], hsrc, start=True, stop=True,
                        perf_mode=DR,
                    )
                    nc.vector.tensor_scalar(
                        hid_sb[:, t, :], hp, b1c[:, t : t + 1], 0.0, OP.add, OP.max
                    )
                l_ps = []
                for t in range(2):
                    lp = pp.tile([P, BS], F32, tag="ps")
                    nc.tensor.matmul(
                        lp, w2t[:, 0:2, ts(t, P)], hid_sb[:, 0:2, :], start=True,
                        stop=False, perf_mode=DR,
                    )
                    nc.tensor.matmul(
                        lp, w2t[:, 2:4, ts(t, P)], hid_sb[:, 2:4, :], start=False,
                        stop=True, perf_mode=DR,
                    )
                    l_ps.append(lp)
                slot = m % (2 * SUB)
                strm_m = strm_t.pop(m)
                prods = prodp.tile([P, 2, BS], FP8, tag="prod")
                for k in range(2):
                    nc.vector.tensor_copy(out=lbuf[:, k, slot, :], in_=l_ps[k])
                    nc.vector.scalar_tensor_tensor(
                        prods[:, k, :], l_ps[k], b2c[:, k : k + 1],
                        strm_m[:, k, :], OP.add, OP.mult,
                    )
                if (m + 1) % SUB == 0 and SUB <= m + 1 < n_steps:
                    defer_block(m + 1 - SUB)
                return (m, prods)

            # ---- scan ------------------------------------------------------
            # Iteration i: h-independent PE work, then the MLP of step i-1
            # (inputs ready), then gates/cell of step i. The last step's
            # gates/cell are skipped (the reference discards the final carry).
            for i in range(n_steps):
                if i + 1 < n_steps - 1:
                    x_t[i + 1] = gather_x(i + 1)
                if prev_prod is not None:
                    pj, pprod = prev_prod
                    nc.tensor.matmul(
                        pick_ps,
                        slide4[:, :, D - 1 - pj : 2 * D - 1 - pj],
                        pprod,
                        start=(pj == 0),
                        stop=False,
                        perf_mode=DR,
                        skip_group_check=True,
                    )
                    prev_prod = None
                pop_esum(1)
                if i + 4 < n_steps:
                    fetch_stream(i + 4)

                if i >= 1:
                    prev_prod = mlp_step(i - 1)

                if i < n_steps - 1:
                    gt = [None] * 8
                    xi = x_t.pop(i)
                    hsrc = h_t[i % 2]
                    hdst = h_t[(i + 1) % 2]
                    for j in (0, 4, 2, 6):
                        gt[j] = gate(j, xi, hsrc, with_h=True)
                    cell_k(gt, 0, hdst)
                    for j in (1, 5, 3, 7):
                        gt[j] = gate(j, xi, hsrc, with_h=True)
                    cell_k(gt, 1, hdst)

            # final picks (steps n-2, n-1) + last deferred block + drain
            pj, pprod = prev_prod
            nc.tensor.matmul(
                pick_ps,
                slide4[:, :, D - 1 - pj : 2 * D - 1 - pj],
                pprod,
                start=False,
                stop=False,
                perf_mode=DR,
                skip_group_check=True,
            )
            pj, pprod = mlp_step(n_steps - 1)
            nc.tensor.matmul(
                pick_ps,
                slide4[:, :, D - 1 - pj : 2 * D - 1 - pj],
                pprod,
                start=False,
                stop=True,
                perf_mode=DR,
                skip_group_check=True,
            )
            defer_block(n_steps - SUB)
            pop_esum(len(pending_esum))

            # ---- epilogue: out = sum_j (pick_j - ln(esum_j)) ----------------
            ln_e = sing.tile([D, BS], F32, tag="lne")
            nc.scalar.activation(ln_e, esum_ps, AF.Ln)
            diff = sing.tile([D, BS], F32, tag="diff")
            nc.vector.tensor_sub(diff, pick_ps, ln_e)
            fin_ps = pp.tile([1, BS], F32, tag="ps")
            nc.tensor.matmul(fin_ps, ones64[:, 0:1], diff, start=True, stop=True)
            out_sb = sing.tile([1, BS], F32, tag="outsb")
            nc.scalar.activation(out_sb, fin_ps, AF.Copy)
            nc.sync.dma_start(out_d, out_sb)

    nc.compile()
    return nc


def prep_inputs(token_embed, W_ih, b_ih, W_hh, b_hh, W1, b1, W2, b2, pos_list,
                input_samples):
    """Host-side layout prep -> per-core in_maps for run_bass_kernel_spmd."""
    f = np.float32
    slide = np.zeros((P, 2, 2 * D), f)
    slide[:, :, D - 1] = 1.0
    shared = {
        "wiht": np.ascontiguousarray(np.asarray(W_ih, f).T).astype(NPE4),
        "whht": np.ascontiguousarray(np.asarray(W_hh, f).T).astype(NPE4),
        "w1t": np.ascontiguousarray(np.asarray(W1, f).T).astype(NPE4),
        "w2t": np.ascontiguousarray(np.asarray(W2, f).T).astype(NPE4),
        "te": np.asarray(token_embed, f).astype(NPE4),
        "bgc": np.ascontiguousarray(
            (np.asarray(b_ih, f) + np.asarray(b_hh, f)).reshape(8, P).T
        ),
        "b1c": np.ascontiguousarray(np.asarray(b1, f).reshape(4, P).T),
        "b2c": np.ascontiguousarray(np.asarray(b2, f).reshape(2, P).T),
        "slide4": slide.astype(NPE4),
        "slide5": slide.astype(NPE5),
        "ones64": np.ones((D, 1), f),
    }
    pe_e4 = _pe_table().astype(NPE4)  # quantize the table once, then gather
    samples = np.asarray(input_samples)
    poss = np.asarray(pos_list)
    didx = np.arange(D)[:, None]
    bidx = np.arange(BS)[None, :]
    in_maps = []
    for c in range(NCORES):
        lo, hi = c * BS, (c + 1) * BS
        sT = np.ascontiguousarray(samples[lo:hi].T)  # [D, BS]
        pT = np.ascontiguousarray(poss[lo:hi].T)
        strm = np.zeros((D, P, 4, BS), NPE4)
        strm[didx, sT % P, sT // P, bidx] = 1.0  # one-hot blocks 0:2
        # pex blocks 2:4: strm[i, p, 2+k, b] = pe_table[pos[b, i], 128k + p]
        strm[:, :, 2:4, :] = pe_e4[pT].reshape(D, BS, 2, P).transpose(0, 3, 2, 1)
        m = dict(shared)
        m["strm"] = strm
        in_maps.append(m)
    return in_maps


_CACHE = {}


def kernel(**inputs) -> np.ndarray:
    if "nc" not in _CACHE:
        _CACHE["nc"] = build_bass()
    nc = _CACHE["nc"]
    in_maps = prep_inputs(**inputs)
    res = bass_utils.run_bass_kernel_spmd(nc, in_maps, core_ids=list(range(NCORES)))
    _CACHE["last_results"] = res
    out = np.empty((B, 1), np.float32)
    for c in range(NCORES):
        out[c * BS : (c + 1) * BS, 0] = np.asarray(
            res.results[c]["out"], np.float32
        ).reshape(BS)
    return out


# revision 20
# speedup vs baseline: 1.6051x; 1.0763x over previous
"""Trainium2 Bass kernel for nn_CondRnnSampler.

Computes, for each batch row b:
    out[b] = sum_i log_softmax(MLP(h_i))[s_i]  over a 64-step LSTM scan,
with the LSTM consuming x_i = token_embed[s_i] + pos_enc(p_i).

Strategy: pure data parallel over 8 NeuronCores (512 batch rows each).
All activations are feature-major ([features-on-partitions, batch-on-free]) so
every layer is a stationary-weight matmul with N=512 moving columns. All large
matmuls run in fp8e4 with MatmulPerfMode.DoubleRow: both 128-row k-tiles of the
contraction are packed into one instruction ([K=128, 2, M] stationary against
[K=128, 2, N] moving), halving the PE instruction count vs bf16. Accumulation
stays fp32 in PSUM; the LSTM cell state c is bf16.

Per step one merged DMA delivers the host-built index encodings: the sample
one-hot (fp8, DoubleRow-packed) and the positional-encoding rows pe_table[p]
(fp8, same packing) in a single [128, 4, 512] tile. token_embed[s] is a
one-hot DoubleRow matmul; x = (te gather PSUM) + pex in one fused DVE add.
The per-step picked-logit and softmax-denominator reductions are single
DoubleRow matmuls against a sliding selector so step j lands on PSUM partition
j; exp is deferred into SUB-step blocks (one big ACTIVATE per k-tile, output
fp8e5 for range) so the ACT table set switches only twice per SUB steps.

Step order follows the proven baseline: all consumers of h_i (x-gather for
i+1, previous pick, esum drain, hid, gates, logits) are emitted before the
cell update writes h_{i+1}, keeping the recurrence chain short and the
h-independent work in front of it.
"""

import sys

sys.path.insert(0, "/opt/trn_rl_repo")

from contextlib import ExitStack

import ml_dtypes
import numpy as np

import concourse.bacc as bacc
import concourse.tile as tile
from concourse import bass_utils, mybir
from concourse.bass import ts

B, D, E, NCL = 4096, 64, 256, 256  # batch, steps, embed, n_choices
NCORES = 8
BS = B // NCORES  # 512 rows per core
P = 128
SUB = 8  # deferred-softmax block (steps)

AF = mybir.ActivationFunctionType
OP = mybir.AluOpType
F32 = mybir.dt.float32
BF16 = mybir.dt.bfloat16
FP8 = mybir.dt.float8e4
FP8E5 = mybir.dt.float8e5
DR = mybir.MatmulPerfMode.DoubleRow

NPBF = ml_dtypes.bfloat16
NPE4 = ml_dtypes.float8_e4m3
NPE5 = ml_dtypes.float8_e5m2

SIG = AF.Sigmoid
TANH = AF.Tanh
# gate-dim blocks j of 128 over 4E=1024: (i0,i1,f0,f1,g0,g1,o0,o1)
GATE_FUNCS = [SIG, SIG, SIG, SIG, TANH, TANH, SIG, SIG]


def _pe_table() -> np.ndarray:
    half = np.float32(E // 2)
    inv = (
        np.float32(1.0)
        / (np.float32(10000.0) ** (np.arange(E // 2, dtype=np.float32) / half))
    ).astype(np.float32)
    pos = np.arange(D, dtype=np.float32)[:, None]
    ang = pos * inv[None, :]
    return np.concatenate([np.sin(ang), np.cos(ang)], axis=1).astype(np.float32)


def build_bass(n_steps: int = D):
    """Build the per-core Bass program (identical on all 8 cores)."""
    nc = bacc.Bacc("TRN2", debug=False, target_bir_lowering=False, num_devices=NCORES)

    def din(name, shape, dt=FP8):
        return nc.dram_tensor(name, list(shape), dt, kind="ExternalInput").ap()

    wiht_d = din("wiht", (E, 4 * E))  # W_ih.T
    whht_d = din("whht", (E, 4 * E))  # W_hh.T
    w1t_d = din("w1t", (E, 2 * E))  # W1.T
    w2t_d = din("w2t", (2 * E, NCL))  # W2.T
    te_d = din("te", (NCL, E))  # token_embed (lhsT for the gather)
    bgc_d = din("bgc", (P, 8), F32)
    b1c_d = din("b1c", (P, 4), F32)
    b2c_d = din("b2c", (P, 2), F32)
    # per-step stream: blocks (oh_k0, oh_k1, pex_k0, pex_k1)
    strm_d = din("strm", (D, P, 4, BS))
    slide4_d = din("slide4", (P, 2, 2 * D))  # pick selector (fp8e4)
    slide5_d = din("slide5", (P, 2, 2 * D), FP8E5)  # esum selector
    ones64_d = din("ones64", (D, 1), F32)
    out_d = nc.dram_tensor("out", [1, BS], F32, kind="ExternalOutput").ap()

    with tile.TileContext(nc) as tc:
        with ExitStack() as ctx:
            sing = ctx.enter_context(tc.tile_pool(name="sing", bufs=1))
            strmq = ctx.enter_context(tc.tile_pool(name="strmq", bufs=6))
            xpool = ctx.enter_context(tc.tile_pool(name="xpool", bufs=3))
            gpool = ctx.enter_context(tc.tile_pool(name="gpool", bufs=10))
            tpool = ctx.enter_context(tc.tile_pool(name="tpool", bufs=4))
            hidp = ctx.enter_context(tc.tile_pool(name="hidp", bufs=2))
            prodp = ctx.enter_context(tc.tile_pool(name="prodp", bufs=3))
            epool = ctx.enter_context(tc.tile_pool(name="epool", bufs=2))
            pp = ctx.enter_context(tc.tile_pool(name="pp", bufs=4, space="PSUM"))
            xpp = ctx.enter_context(tc.tile_pool(name="xpp", bufs=1, space="PSUM"))
            psing = ctx.enter_context(
                tc.tile_pool(name="psing", bufs=1, space="PSUM")
            )

            # ---- streaming index-derived inputs -----------------------------
            strm_t = {}

            def fetch_stream(i):
                s = strmq.tile([P, 4, BS], FP8, tag="strm")
                nc.sync.dma_start(s[:], strm_d[i])
                strm_t[i] = s

            for _i in range(4):
                fetch_stream(_i)

            # ---- resident SBUF tensors -------------------------------------
            te2 = sing.tile([P, 2, E], FP8, tag="te")
            nc.sync.dma_start(te2[:], te_d.rearrange("(ko p) m -> p ko m", p=P))
            wiht = sing.tile([P, 2, 4 * E], FP8, tag="wiht")
            nc.sync.dma_start(wiht[:], wiht_d.rearrange("(ko p) m -> p ko m", p=P))
            bgc = sing.tile([P, 8], F32, tag="bgc")
            nc.sync.dma_start(bgc[:], bgc_d)
            whht = sing.tile([P, 2, 4 * E], FP8, tag="whht")
            nc.sync.dma_start(whht[:], whht_d.rearrange("(ko p) m -> p ko m", p=P))
            w1t = sing.tile([P, 2, 2 * E], FP8, tag="w1t")
            nc.sync.dma_start(w1t[:], w1t_d.rearrange("(ko p) m -> p ko m", p=P))
            w2t = sing.tile([P, 4, NCL], FP8, tag="w2t")
            nc.sync.dma_start(w2t[:], w2t_d.rearrange("(ko p) m -> p ko m", p=P))
            b1c = sing.tile([P, 4], F32, tag="b1c")
            nc.sync.dma_start(b1c[:], b1c_d)
            b2c = sing.tile([P, 2], F32, tag="b2c")
            nc.sync.dma_start(b2c[:], b2c_d)
            slide4 = sing.tile([P, 2, 2 * D], FP8, tag="slide4")
            nc.sync.dma_start(slide4[:], slide4_d)
            slide5 = sing.tile([P, 2, 2 * D], FP8E5, tag="slide5")
            nc.sync.dma_start(slide5[:], slide5_d)
            ones64 = sing.tile([D, 1], F32, tag="ones64")
            nc.sync.dma_start(ones64[:], ones64_d)

            h0_sb = sing.tile([P, 2, BS], FP8, tag="h0")
            h1_sb = sing.tile([P, 2, BS], FP8, tag="h1")
            h_t = [h0_sb, h1_sb]
            c_sb = sing.tile([P, 2, BS], BF16, tag="c")
            lbuf = sing.tile([P, 2, 2 * SUB, BS], BF16, tag="lbuf")
            esum_ps = psing.tile([D, BS], F32, tag="esum")
            pick_ps = psing.tile([D, BS], F32, tag="pick")

            def gate(j, x_sb, hsrc, with_h):
                """Pre-act matmuls + activation for gate-dim block j (of 8)."""
                g_ps = pp.tile([P, BS], F32, tag="ps")
                nc.tensor.matmul(
                    g_ps, wiht[:, :, ts(j, P)], x_sb, start=True, stop=not with_h,
                    perf_mode=DR,
                )
                if with_h:
                    nc.tensor.matmul(
                        g_ps, whht[:, :, ts(j, P)], hsrc, start=False, stop=True,
                        perf_mode=DR,
                    )
                g_sb = gpool.tile([P, BS], BF16, tag="gate")
                nc.scalar.activation(g_sb, g_ps, GATE_FUNCS[j], bias=bgc[:, j : j + 1])
                return g_sb

            def cell_k(gt, k, hdst):
                """c[:,k] = f*c + i*g; h[:,k] = o*tanh(c). gt indexed by block j.

                i*g runs on GPSIMD in parallel with f*c on DVE; the rest of the
                recurrence-critical chain (add, tanh, h mul) stays on DVE/ACT.
                """
                ig = tpool.tile([P, BS], BF16, tag="tmp")
                nc.gpsimd.tensor_mul(ig, gt[0 + k], gt[4 + k])
                fc = tpool.tile([P, BS], BF16, tag="tmp")
                nc.vector.tensor_mul(fc, gt[2 + k], c_sb[:, k, :])
                nc.vector.tensor_add(c_sb[:, k, :], ig, fc)
                tcl = tpool.tile([P, BS], BF16, tag="tc")
                nc.scalar.activation(tcl, c_sb[:, k, :], TANH)
                nc.vector.tensor_mul(hdst[:, k, :], gt[6 + k], tcl)

            pending_esum = []  # FIFO of deferred esum matmuls

            def defer_block(j0):
                """exp for steps j0..j0+SUB-1: one big ACTIVATE per k-tile."""
                blk = (j0 // SUB) % 2
                e = epool.tile([P, SUB, 2, BS], FP8E5, tag="e")
                for k in range(2):
                    nc.scalar.activation(
                        e[:, :, k, :],
                        lbuf[:, k, blk * SUB : (blk + 1) * SUB, :],
                        AF.Exp,
                        bias=b2c[:, k : k + 1],
                    )
                for j in range(j0, j0 + SUB):
                    pending_esum.append((j, e))

            def pop_esum(n):
                for _ in range(min(n, len(pending_esum))):
                    j, e = pending_esum.pop(0)
                    nc.tensor.matmul(
                        esum_ps,
                        slide5[:, :, D - 1 - j : 2 * D - 1 - j],
                        e[:, j % SUB, :, :],
                        start=(j == 0),
                        stop=(j == n_steps - 1),
                        perf_mode=DR,
                        skip_group_check=True,
                    )

            def gather_x(i):
                """x_i = token_embed[s_i] + pe_rows, fp8 [P, 2, BS] in SBUF."""
                strm = strm_t[i]
                x_ps = xpp.tile([P, 2, BS], F32, tag="xps")
                for t in range(2):
                    nc.tensor.matmul(
                        x_ps[:, t, :], te2[:, :, ts(t, P)], strm[:, 0:2, :],
                        start=True, stop=True, perf_mode=DR,
                    )
                x_sb = xpool.tile([P, 2, BS], FP8, tag="x")
                nc.vector.tensor_add(x_sb, x_ps, strm[:, 2:4, :])
                return x_sb

            # ---- init: h,c from lstm(pe[:,0]) with zero state ---------------
            # x0 is exactly the pex half of stream 0 (fp8, DoubleRow-packed).
            gt0 = [None] * 8
            for j in (0, 4, 2, 6, 1, 5, 3, 7):
                gt0[j] = gate(j, strm_t[0][:, 2:4, :], None, with_h=False)
            for k in range(2):
                nc.vector.tensor_mul(c_sb[:, k, :], gt0[0 + k], gt0[4 + k])
                tcl = tpool.tile([P, BS], BF16, tag="tc")
                nc.scalar.activation(tcl, c_sb[:, k, :], TANH)
                nc.vector.tensor_mul(h_t[0][:, k, :], gt0[6 + k], tcl)

            x_t = {0: gather_x(0)}
            prev_prod = None  # (step, prod pair tile) awaiting the pick matmul

            def mlp_step(m):
                """hid/logits/logit-stash/pick-product for step m.

                Runs one iteration behind the recurrence: reads h_t[m % 2],
                which was finalized in the previous iteration, so every input
                is ready at iteration start and the PE/DVE queues never wait
                on the fresh h.
                """
                hsrc = h_t[m % 2]
                hid_sb = hidp.tile([P, 4, BS], FP8, tag="hid")
                for t in range(4):
                    hp = pp.tile([P, BS], F32, tag="ps")
                    nc.tensor.matmul(
                        hp, w1t[:, :, ts(t, P)], hsrc, start=True, stop=True,
                        perf_mode=DR,
                    )
                    if t == 3:
                        nc.scalar.activation(
                            hid_sb[:, t, :], hp, AF.Relu, bias=b1c[:, t : t + 1]
                        )
                    else:
                        nc.vector.tensor_scalar(
                            hid_sb[:, t, :], hp, b1c[:, t : t + 1], 0.0,
                            OP.add, OP.max,
                        )
                l_ps = []
                for t in range(2):
                    lp = pp.tile([P, BS], F32, tag="ps")
                    nc.tensor.matmul(
                        lp, w2t[:, 0:2, ts(t, P)], hid_sb[:, 0:2, :], start=True,
                        stop=False, perf_mode=DR,
                    )
                    nc.tensor.matmul(
                        lp, w2t[:, 2:4, ts(t, P)], hid_sb[:, 2:4, :], start=False,
                        stop=True, perf_mode=DR,
                    )
                    l_ps.append(lp)
                slot = m % (2 * SUB)
                strm_m = strm_t.pop(m)
                prods = prodp.tile([P, 2, BS], FP8, tag="prod")
                for k in range(2):
                    nc.vector.tensor_copy(out=lbuf[:, k, slot, :], in_=l_ps[k])
                    nc.vector.scalar_tensor_tensor(
                        prods[:, k, :], l_ps[k], b2c[:, k : k + 1],
                        strm_m[:, k, :], OP.add, OP.mult,
                    )
                if (m + 1) % SUB == 0 and SUB <= m + 1 < n_steps:
                    defer_block(m + 1 - SUB)
                return (m, prods)

            # ---- scan ------------------------------------------------------
            # Iteration i: h-independent PE work, then the MLP of step i-1
            # (inputs ready), then gates/cell of step i. The last step's
            # gates/cell are skipped (the reference discards the final carry).
            for i in range(n_steps):
                if i + 1 < n_steps - 1:
                    x_t[i + 1] = gather_x(i + 1)
                if prev_prod is not None:
                    pj, pprod = prev_prod
                    nc.tensor.matmul(
                        pick_ps,
                        slide4[:, :, D - 1 - pj : 2 * D - 1 - pj],
                        pprod,
                        start=(pj == 0),
                        stop=False,
                        perf_mode=DR,
                        skip_group_check=True,
                    )
                    prev_prod = None
                pop_esum(1)
                if i + 4 < n_steps:
                    fetch_stream(i + 4)

                if i >= 1:
                    prev_prod = mlp_step(i - 1)

                if i < n_steps - 1:
                    gt = [None] * 8
                    xi = x_t.pop(i)
                    hsrc = h_t[i % 2]
                    hdst = h_t[(i + 1) % 2]
                    for j in (0, 4, 2, 6):
                        gt[j] = gate(j, xi, hsrc, with_h=True)
                    cell_k(gt, 0, hdst)
                    for j in (1, 5, 3, 7):
                        gt[j] = gate(j, xi, hsrc, with_h=True)
                    cell_k(gt, 1, hdst)

            # final picks (steps n-2, n-1) + last deferred block + drain
            pj, pprod = prev_prod
            nc.tensor.matmul(
                pick_ps,
                slide4[:, :, D - 1 - pj : 2 * D - 1 - pj],
                pprod,
                start=False,
                stop=False,
                perf_mode=DR,
                skip_group_check=True,
            )
            pj, pprod = mlp_step(n_steps - 1)
            nc.tensor.matmul(
                pick_ps,
                slide4[:, :, D - 1 - pj : 2 * D - 1 - pj],
                pprod,
                start=False,
                stop=True,
                perf_mode=DR,
                skip_group_check=True,
            )
            defer_block(n_steps - SUB)
            pop_esum(len(pending_esum))

            # ---- epilogue: out = sum_j (pick_j - ln(esum_j)) ----------------
            ln_e = sing.tile([D, BS], F32, tag="lne")
            nc.scalar.activation(ln_e, esum_ps, AF.Ln)
            diff = sing.tile([D, BS], F32, tag="diff")
            nc.vector.tensor_sub(diff, pick_ps, ln_e)
            fin_ps = pp.tile([1, BS], F32, tag="ps")
            nc.tensor.matmul(fin_ps, ones64[:, 0:1], diff, start=True, stop=True)
            out_sb = sing.tile([1, BS], F32, tag="outsb")
            nc.scalar.activation(out_sb, fin_ps, AF.Copy)
            nc.sync.dma_start(out_d, out_sb)

    nc.compile()
    return nc


def prep_inputs(token_embed, W_ih, b_ih, W_hh, b_hh, W1, b1, W2, b2, pos_list,
                input_samples):
    """Host-side layout prep -> per-core in_maps for run_bass_kernel_spmd."""
    f = np.float32
    slide = np.zeros((P, 2, 2 * D), f)
    slide[:, :, D - 1] = 1.0
    shared = {
        "wiht": np.ascontiguousarray(np.asarray(W_ih, f).T).astype(NPE4),
        "whht": np.ascontiguousarray(np.asarray(W_hh, f).T).astype(NPE4),
        "w1t": np.ascontiguousarray(np.asarray(W1, f).T).astype(NPE4),
        "w2t": np.ascontiguousarray(np.asarray(W2, f).T).astype(NPE4),
        "te": np.asarray(token_embed, f).astype(NPE4),
        "bgc": np.ascontiguousarray(
            (np.asarray(b_ih, f) + np.asarray(b_hh, f)).reshape(8, P).T
        ),
        "b1c": np.ascontiguousarray(np.asarray(b1, f).reshape(4, P).T),
        "b2c": np.ascontiguousarray(np.asarray(b2, f).reshape(2, P).T),
        "slide4": slide.astype(NPE4),
        "slide5": slide.astype(NPE5),
        "ones64": np.ones((D, 1), f),
    }
    pe_e4 = _pe_table().astype(NPE4)  # quantize the table once, then gather
    samples = np.asarray(input_samples)
    poss = np.asarray(pos_list)
    didx = np.arange(D)[:, None]
    bidx = np.arange(BS)[None, :]
    in_maps = []
    for c in range(NCORES):
        lo, hi = c * BS, (c + 1) * BS
        sT = np.ascontiguousarray(samples[lo:hi].T)  # [D, BS]
        pT = np.ascontiguousarray(poss[lo:hi].T)
        strm = np.zeros((D, P, 4, BS), NPE4)
        strm[didx, sT % P, sT // P, bidx] = 1.0  # one-hot blocks 0:2
        # pex blocks 2:4: strm[i, p, 2+k, b] = pe_table[pos[b, i], 128k + p]
        strm[:, :, 2:4, :] = pe_e4[pT].reshape(D, BS, 2, P).transpose(0, 3, 2, 1)
        m = dict(shared)
        m["strm"] = strm
        in_maps.append(m)
    return in_maps


_CACHE = {}


def kernel(**inputs) -> np.ndarray:
    if "nc" not in _CACHE:
        _CACHE["nc"] = build_bass()
    nc = _CACHE["nc"]
    in_maps = prep_inputs(**inputs)
    res = bass_utils.run_bass_kernel_spmd(nc, in_maps, core_ids=list(range(NCORES)))
    _CACHE["last_results"] = res
    out = np.empty((B, 1), np.float32)
    for c in range(NCORES):
        out[c * BS : (c + 1) * BS, 0] = np.asarray(
            res.results[c]["out"], np.float32
        ).reshape(BS)
    return out
